# revision 5
# baseline (speedup 1.0000x reference)
"""Trainium2 Bass kernel for nn_MixedLinearV2_InProj_MHA.

Math: out[b,s,o] = sum_i x[b,s,i] * (W*M)[o,i] + b_mix[o], where
M = sum_k weights[k] * row_mask_k x col_mask_k is a 3x3 block-constant
mixing mask and b_mix = b * (weights @ row_mask). Rows o >= 3072 of both
W*M and b_mix are exactly zero (max out_dim is 3072 < 6144), so the top
half of the output is zero and only a [8192,1024]x[1024,3072] matmul
remains.

Strategy: data-parallel over the batch dim (8 cores x 1 batch each).
Host folds the scalar mixing mask into W (cheap: 3M elements); each
core computes x_c @ W_mix.T + b_mix with fp32r matmuls (full PE rate,
~1.5e-4 scale-relative error vs fp32). Per 128-row output stripe, 6
PSUM banks accumulate over 8 K-chunks; the bias enters PSUM via a K=1
ones-matmul on the PE, so the PSUM drain is a plain copy that
alternates between VectorE and ScalarE and stays off the critical
path. Steady-state measured ~89 us/core vs an 82 us fp32r PE floor.
"""

import numpy as np

import concourse.bass as bass
import concourse.mybir as mybir
import concourse.tile as tile
from concourse import bacc
from concourse.bass_utils import run_bass_kernel_spmd

B, S, DIN = 8, 1024, 1024
DOUT_FULL = 6144
DOUT = 3072            # rows beyond this are exactly zero
N_CORES = 8
NT = 512               # matmul moving-dim tile (one fp32 PSUM bank)
OB = DOUT // NT        # 6 o-tiles per stripe
KC = DIN // 128        # 8 contraction chunks
ST = S // 128          # 8 output row stripes per core

# combo k=(embed,heads): out_dims/in_dims of the zero-padded weight slices
OUT_DIMS = (1536, 1536, 2304, 2304, 3072, 3072)
IN_DIMS = (512, 512, 768, 768, 1024, 1024)

F32R = mybir.dt.float32r
F32 = mybir.dt.float32

_cache = {}


def _build(iters=1):
    """Compile the per-core program. `iters` repeats the whole body for
    on-device timing via differencing (the graded path uses iters=1)."""
    if iters in _cache:
        return _cache[iters]

    nc = bacc.Bacc("TRN2", target_bir_lowering=False, debug=False,
                   num_devices=N_CORES)
    xT_d = nc.dram_tensor("xT", [DIN, S], F32R, kind="ExternalInput")
    wT_d = nc.dram_tensor("wT", [DIN, DOUT], F32R, kind="ExternalInput")
    bias_d = nc.dram_tensor("bias", [1, DOUT], F32R, kind="ExternalInput")
    ones_d = nc.dram_tensor("ones", [1, 128], F32R, kind="ExternalInput")
    o_d = nc.dram_tensor("o", [S, DOUT], F32, kind="ExternalOutput")

    with tile.TileContext(nc) as tc:
        with (
            tc.tile_pool(name="res", bufs=1) as res_pool,
            tc.tile_pool(name="outp", bufs=3) as out_pool,
            tc.tile_pool(name="psum", bufs=8,
                         space=bass.MemorySpace.PSUM) as psum_pool,
        ):
            for it in range(iters):
                bias = res_pool.tile([1, DOUT], F32R, tag="bias",
                                     name=f"bias_{it}")
                nc.sync.dma_start(bias[:], bias_d.ap()[:, :])
                ones = res_pool.tile([1, 128], F32R, tag="ones",
                                     name=f"ones_{it}")
                nc.sync.dma_start(ones[:], ones_d.ap()[:, :])
                # interleave (x[i], w[i]) loads: stripe 0's i-th matmul
                # step only needs chunk i, so compute starts after the
                # first pair instead of after the full 16 MB
                xt, wt = [], []
                for i in range(KC):
                    t = res_pool.tile([128, S], F32R, tag=f"x{i}",
                                      name=f"x_{it}_{i}")
                    nc.sync.dma_start(t[:], xT_d.ap()[i * 128:(i + 1) * 128, :])
                    xt.append(t)
                    u = res_pool.tile([128, DOUT], F32R, tag=f"w{i}",
                                      name=f"w_{it}_{i}")
                    nc.sync.dma_start(u[:], wT_d.ap()[i * 128:(i + 1) * 128, :])
                    wt.append(u)
                for s in range(ST):
                    accs = [psum_pool.tile([128, NT], F32, tag="ps",
                                           name=f"ps_{it}_{s}_{ob}")
                            for ob in range(OB)]
                    for i in range(KC):
                        lhsT = xt[i][:, s * 128:(s + 1) * 128]
                        for ob in range(OB):
                            nc.tensor.matmul(
                                accs[ob][:], lhsT,
                                wt[i][:, ob * NT:(ob + 1) * NT],
                                start=(i == 0), stop=False)
                    for ob in range(OB):
                        nc.tensor.matmul(
                            accs[ob][:], ones[:],
                            bias[:, ob * NT:(ob + 1) * NT],
                            start=False, stop=True)
                    outt = out_pool.tile([128, DOUT], F32, tag="out",
                                         name=f"out_{it}_{s}")
                    for ob in range(OB):
                        sl = outt[:, ob * NT:(ob + 1) * NT]
                        if ob % 2 == 0:
                            nc.vector.tensor_copy(sl, accs[ob][:])
                        else:
                            nc.scalar.copy(sl, accs[ob][:])
                    nc.sync.dma_start(o_d.ap()[s * 128:(s + 1) * 128, :],
                                      outt[:])

    nc.compile()
    _cache[iters] = nc
    return nc


def _prep(x, weights, W, b):
    """Host-side: fold the mixing mask into W, build per-core inputs."""
    w = np.asarray(weights, dtype=np.float32)
    M = np.zeros((DOUT, DIN), dtype=np.float32)
    rowsum = np.zeros(DOUT, dtype=np.float32)
    for k in range(6):
        M[:OUT_DIMS[k], :IN_DIMS[k]] += w[k]
        rowsum[:OUT_DIMS[k]] += w[k]
    WmT = np.ascontiguousarray((np.asarray(W[:DOUT], np.float32) * M).T)
    b_mix = (np.asarray(b[:DOUT], np.float32) * rowsum).reshape(1, DOUT)
    ones = np.ones((1, 128), np.float32)

    in_maps = []
    for c in range(N_CORES):
        xT = np.ascontiguousarray(np.asarray(x[c], np.float32).T)
        in_maps.append({"xT": xT, "wT": WmT, "bias": b_mix, "ones": ones})
    return in_maps


def _run(in_maps, iters=1):
    nc = _build(iters)
    return run_bass_kernel_spmd(nc, in_maps, list(range(N_CORES)))


def kernel(x, weights, W, b):
    in_maps = _prep(x, weights, W, b)
    res = _run(in_maps)
    out = np.zeros((B, S, DOUT_FULL), dtype=np.float32)
    for c in range(N_CORES):
        out[c, :, :DOUT] = res.results[c]["o"]
    return out


# revision 6
# speedup vs baseline: 1.4162x; 1.4162x over previous
"""Trainium2 Bass kernel for nn_MixedLinearV2_InProj_MHA.

Math: out[b,s,o] = sum_i x[b,s,i] * (W*M)[o,i] + b_mix[o], where
M = sum_k weights[k] * row_mask_k x col_mask_k is a 3x3 block-constant
mixing mask and b_mix = b * (weights @ row_mask). Rows o >= 3072 of both
W*M and b_mix are exactly zero (max out_dim is 3072 < 6144), so the top
half of the output is zero and only a [8192,1024]x[1024,3072] matmul
remains.

Strategy: data-parallel over the batch dim (8 cores x 1 batch each).
Host folds the scalar mixing mask into W (cheap: 3M elements); each
core computes x_c @ W_mix.T + b_mix with fp32r matmuls (full PE rate,
~1.5e-4 scale-relative error vs fp32). Per 128-row output stripe, 6
PSUM banks accumulate over 8 K-chunks; the bias enters PSUM via a K=1
ones-matmul on the PE, so the PSUM drain is a plain copy that
alternates between VectorE and ScalarE and stays off the critical
path. Steady-state measured ~89 us/core vs an 82 us fp32r PE floor.
"""

import numpy as np

import concourse.bass as bass
import concourse.mybir as mybir
import concourse.tile as tile
from concourse import bacc
from concourse.bass_utils import run_bass_kernel_spmd

B, S, DIN = 8, 1024, 1024
DOUT_FULL = 6144
DOUT = 3072            # rows beyond this are exactly zero
N_CORES = 8
NT = 512               # matmul moving-dim tile (one fp32 PSUM bank)
OB = DOUT // NT        # 6 o-tiles per stripe
KC = DIN // 128        # 8 contraction chunks
ST = S // 128          # 8 output row stripes per core

# combo k=(embed,heads): out_dims/in_dims of the zero-padded weight slices
OUT_DIMS = (1536, 1536, 2304, 2304, 3072, 3072)
IN_DIMS = (512, 512, 768, 768, 1024, 1024)

F32R = mybir.dt.float32r
F32 = mybir.dt.float32

_cache = {}


def _build(iters=1):
    """Compile the per-core program. `iters` repeats the whole body for
    on-device timing via differencing (the graded path uses iters=1)."""
    if iters in _cache:
        return _cache[iters]

    nc = bacc.Bacc("TRN2", target_bir_lowering=False, debug=False,
                   num_devices=N_CORES)
    xT_d = nc.dram_tensor("xT", [DIN, S], F32R, kind="ExternalInput")
    wT_d = nc.dram_tensor("wT", [DIN, DOUT], F32R, kind="ExternalInput")
    bias_d = nc.dram_tensor("bias", [1, DOUT], F32R, kind="ExternalInput")
    ones_d = nc.dram_tensor("ones", [1, 128], F32R, kind="ExternalInput")
    o_d = nc.dram_tensor("o", [S, DOUT], F32, kind="ExternalOutput")

    with tile.TileContext(nc) as tc:
        with (
            tc.tile_pool(name="res", bufs=1) as res_pool,
            tc.tile_pool(name="outp", bufs=3) as out_pool,
            tc.tile_pool(name="psum", bufs=8,
                         space=bass.MemorySpace.PSUM) as psum_pool,
        ):
            for it in range(iters):
                bias = res_pool.tile([1, DOUT], F32R, tag="bias",
                                     name=f"bias_{it}")
                nc.sync.dma_start(bias[:], bias_d.ap()[:, :])
                ones = res_pool.tile([1, 128], F32R, tag="ones",
                                     name=f"ones_{it}")
                nc.sync.dma_start(ones[:], ones_d.ap()[:, :])
                # grouped loads (all x, then all w) measured ~50 us/iter
                # faster steady-state than interleaving x/w pairs
                xt, wt = [], []
                for i in range(KC):
                    t = res_pool.tile([128, S], F32R, tag=f"x{i}",
                                      name=f"x_{it}_{i}")
                    nc.sync.dma_start(t[:], xT_d.ap()[i * 128:(i + 1) * 128, :])
                    xt.append(t)
                for i in range(KC):
                    u = res_pool.tile([128, DOUT], F32R, tag=f"w{i}",
                                      name=f"w_{it}_{i}")
                    nc.sync.dma_start(u[:], wT_d.ap()[i * 128:(i + 1) * 128, :])
                    wt.append(u)
                for s in range(ST):
                    accs = [psum_pool.tile([128, NT], F32, tag="ps",
                                           name=f"ps_{it}_{s}_{ob}")
                            for ob in range(OB)]
                    for i in range(KC):
                        lhsT = xt[i][:, s * 128:(s + 1) * 128]
                        for ob in range(OB):
                            nc.tensor.matmul(
                                accs[ob][:], lhsT,
                                wt[i][:, ob * NT:(ob + 1) * NT],
                                start=(i == 0), stop=False)
                    for ob in range(OB):
                        nc.tensor.matmul(
                            accs[ob][:], ones[:],
                            bias[:, ob * NT:(ob + 1) * NT],
                            start=False, stop=True)
                    outt = out_pool.tile([128, DOUT], F32, tag="out",
                                         name=f"out_{it}_{s}")
                    for ob in range(OB):
                        sl = outt[:, ob * NT:(ob + 1) * NT]
                        if ob % 2 == 0:
                            nc.vector.tensor_copy(sl, accs[ob][:])
                        else:
                            nc.scalar.copy(sl, accs[ob][:])
                    nc.sync.dma_start(o_d.ap()[s * 128:(s + 1) * 128, :],
                                      outt[:])

    nc.compile()
    _cache[iters] = nc
    return nc


def _prep(x, weights, W, b):
    """Host-side: fold the mixing mask into W, build per-core inputs."""
    w = np.asarray(weights, dtype=np.float32)
    M = np.zeros((DOUT, DIN), dtype=np.float32)
    rowsum = np.zeros(DOUT, dtype=np.float32)
    for k in range(6):
        M[:OUT_DIMS[k], :IN_DIMS[k]] += w[k]
        rowsum[:OUT_DIMS[k]] += w[k]
    WmT = np.ascontiguousarray((np.asarray(W[:DOUT], np.float32) * M).T)
    b_mix = (np.asarray(b[:DOUT], np.float32) * rowsum).reshape(1, DOUT)
    ones = np.ones((1, 128), np.float32)

    in_maps = []
    for c in range(N_CORES):
        xT = np.ascontiguousarray(np.asarray(x[c], np.float32).T)
        in_maps.append({"xT": xT, "wT": WmT, "bias": b_mix, "ones": ones})
    return in_maps


def _run(in_maps, iters=1):
    nc = _build(iters)
    return run_bass_kernel_spmd(nc, in_maps, list(range(N_CORES)))


def kernel(x, weights, W, b):
    in_maps = _prep(x, weights, W, b)
    res = _run(in_maps)
    out = np.zeros((B, S, DOUT_FULL), dtype=np.float32)
    for c in range(N_CORES):
        out[c, :, :DOUT] = res.results[c]["o"]
    return out
